# revision 1
# baseline (speedup 1.0000x reference)
"""AntisymmetricRNN Trainium2 kernel — 8-core data-parallel over batch.

Math (per reference):
    mask = strictly-lower-tri; w_r = v_r * mask; A = w_r - w_r.T
    step:  h' = h + (1/TAU) * tanh( tanh(h) @ A + b_r - GAMMA*h )
           x_pred = tanh(h') @ w_o.T + b_o;   err_t = x_pred - x_t

Design (final, ~1586us vs 1767us baseline):
  * batch 256 sharded 8 ways (32 per core); recurrence local per core.
  * state layout "h-major": [128 partitions = h%128, free = (h//128, b)] so
    the recurrent matmul output lands in state layout -> zero transposes.
  * The step period is bound by the dependency ring
        bank-stop -> (drain+sem ~520ns) -> ACT u -> DVE h+= -> ACT th
        -> (sem) -> next step's k-dependent matmuls,
    balanced against the 64-matmul LDWEIGHTS-floor stream (~1.7us).  The
    G=2 layout [m0-3 x k0-3][m0-3 x k4-7][m4-7 x k0-3][m4-7 x k4-7] puts
    bank c0's stop mid-stream so half the tanh work overlaps the stream.
  * chain emitted as u0,h0,u1,h1 then th0,pre0,th1,pre1: ACT queue becomes
    u0,th0,u1,th1 (via _force_chain_order) and DVE queue h0,h1,pre0,pre1,
    so neither prewrite sits between the h-updates on the critical ring.
  * per-chunk TILES (not slices of big tiles): the tile framework tracks
    dependencies at tile granularity.  scratch bufs=1 is deliberate (see
    comment at the pool).
  * output projection runs ONE GROUP BEHIND, spread as 2 k-bursts per step
    into the PE stall window after each step's rec matmuls (a burst = 4
    column-tiled matmuls, tile_position col 32s, sharing one 256-col w_o^T
    stream).  This keeps the projection entirely off the critical ring.
    tanh tiles live in a 12-deep ring for it; xp accumulates 4 steps in
    one [128,256] PSUM tile; one DVE subtract + one contiguous 128KB err
    DMA per group.
  * fully unrolled (no hardware loops).
"""

import numpy as np
import ml_dtypes
from contextlib import ExitStack

import concourse.bass as bass
import concourse.tile as tile
from concourse import mybir
from concourse.bass_utils import run_bass_kernel_spmd

# ---------------- problem constants (hardcoded per spec) ----------------
S, B, D, H = 512, 256, 256, 1024
NCORES = 8
BS = B // NCORES                  # 32 batch per core
TAU, GAMMA = 10.0, 0.1
INV_TAU = 1.0 / TAU
KT = H // 128                     # 8 contraction tiles
MT = H // 128                     # 8 output tiles
G = 2                             # elementwise chunks per step
CW = (MT // G) * BS               # chunk width in free elems (128)
MPQ = MT // G                     # m-tiles per chunk (4)
NSLOT = 4                         # xp accumulation slots per DMA group
NRING = 12                        # tanh ring depth: proj reads a slot up
                                  # to 7 steps after it's written; 12 keeps
                                  # the chain's th writes WAR-free

TRACE = False                     # set True from test harness for profiling
LAST_RESULTS = None               # BassKernelResults stash for the harness

_BUILT = None


def _split_multi_waits(nc, max_waits: int = 1):
    """The walrus build here supports one sync-wait slot on CTRL-encoded
    instructions; split any multi-wait instruction's extra waits into a chain
    of preceding single-wait NOPs on the same engine (identical semantics)."""
    for fn in nc.m.functions:
        for bb in fn.blocks:
            new_insts = []
            for inst in bb.instructions:
                si = inst.sync_info
                if si is not None and len(si.on_wait) > max_waits:
                    waits = list(si.on_wait)
                    for w in waits[:-max_waits]:
                        nop = mybir.InstNoOp(
                            name=nc.get_next_instruction_name(), ins=[], outs=[])
                        nop.engine = inst.engine
                        nop.sync_info = mybir.SyncInfo(on_wait=[w], on_update=[])
                        nc.register_instruction(nop)
                        new_insts.append(nop)
                    si.on_wait = waits[-max_waits:]
                new_insts.append(inst)
            bb.instructions = new_insts


def _force_chain_order(nc):
    """The list scheduler breaks the u1-vs-th0 readiness tie randomly (both
    become ready at h0's completion in its sim), which makes the ACT queue
    order u0,u1,th0,th1 on some builds — parking th0 (which gates the next
    step's first matmuls) behind u1 and costing ~300us over the run.  Force
    u0,th0,u1,th1 deterministically: swap adjacent (u,u,th) -> (u,th,u) on
    the ACT queue and remap every semaphore wait that targeted the two
    swapped completion counts."""
    for fn in nc.m.functions:
        for bb in fn.blocks:
            # ACT-queue instructions that bump the ACT semaphore, in order
            acts = []
            sem_id = None
            for pos, inst in enumerate(bb.instructions):
                if inst.__class__.__name__ != "InstActivation":
                    continue
                si = inst.sync_info
                if si is None:
                    continue
                for u in si.on_update:
                    if u.sync_type == "semaphore":
                        sem_id = u.id if sem_id is None else sem_id
                        assert u.id == sem_id
                acts.append(pos)
            if sem_id is None:
                continue

            def kind(pos):
                mr = bb.instructions[pos].ins[0].memref
                return "u" if mr.startswith("zT") else "th"

            swaps = []          # (count_of_first, count_of_second), 1-based
            i = 0
            while i + 2 < len(acts):
                if (kind(acts[i]) == "u" and kind(acts[i + 1]) == "u"
                        and kind(acts[i + 2]) == "th"):
                    p1, p2 = acts[i + 1], acts[i + 2]
                    bb.instructions[p1], bb.instructions[p2] = (
                        bb.instructions[p2], bb.instructions[p1])
                    swaps.append(i + 2)      # u1 held count i+2, th0 i+3
                    i += 3
                else:
                    i += 1
            if not swaps:
                continue
            swapset = {}
            for c in swaps:
                swapset[c] = c + 1           # waiters on old u1 -> one later
                swapset[c + 1] = c           # waiters on old th0 -> one earlier
            n_remap = 0
            for fn2 in nc.m.functions:
                for bb2 in fn2.blocks:
                    for inst in bb2.instructions:
                        si = inst.sync_info
                        if si is None:
                            continue
                        for w in si.on_wait:
                            if (w.sync_type == "semaphore" and w.id == sem_id
                                    and w.wait_mode == "sem-ge-imm"
                                    and w.wait_value in swapset):
                                w.wait_value = swapset[w.wait_value]
                                n_remap += 1
            assert n_remap > 0




def _sim_makespan(nc):
    """Event-driven replay of the scheduled program using measured hardware
    constants (27ns/MM issue, (N+352)/1.2 ACT, (N+215)/1.2 DVE, ~490/33ns
    sem visibility).  Predicts ~1684us for this kernel (hw: ~1616).  Kept as
    a diagnostic: with _force_chain_order in place, builds hash identically,
    so remaining run-to-run variance (~1616 vs ~1936us) is device state, not
    the schedule.  Unused in the normal path.
    """
    from collections import defaultdict, deque
    qs = defaultdict(deque)
    for fn in nc.m.functions:
        for bb in fn.blocks:
            for inst in bb.instructions:
                qs[str(inst.engine)].append(inst)
    # classify sems: ids with subtract-mode updates (barriers) or where
    # waits exceed the instruction-side total (DMA-hardware-backed) are
    # "external" — identical across builds, model as satisfied early.
    tot = defaultdict(int)
    ext = set()
    maxwait = defaultdict(int)
    for q in qs.values():
        for inst in q:
            si = inst.sync_info
            if si is None:
                continue
            for u in si.on_update:
                if getattr(u, "sync_type", None) == "semaphore":
                    if "sub" in str(getattr(u, "update_mode", "")):
                        ext.add(u.id)
                    else:
                        tot[u.id] += getattr(u, "update_value", None) or 1
            for w in si.on_wait:
                if getattr(w, "sync_type", None) == "semaphore" and w.wait_value:
                    maxwait[w.id] = max(maxwait[w.id], w.wait_value)
    for sid, v in maxwait.items():
        if v > tot[sid]:
            ext.add(sid)
    sem_times = defaultdict(list)          # sem id -> times of each +1 unit
    free = defaultdict(float)              # engine -> next-free time

    def fsz(ap):
        n = 1
        try:
            for pair in list(ap.ap)[1:]:
                n *= pair[1]
        except Exception:
            n = 128
        return n

    def dur_vis(inst):
        c = inst.__class__.__name__
        if c == "InstMatmult":
            return 27.0, 490.0             # issue spacing; drain+sem vis
        if c == "InstLdweights":
            return 0.0, 0.0
        if c == "InstActivation":
            return (fsz(inst.outs[0]) + 352) / 1.2, 33.0
        if c in ("InstTensorScalarPtr", "InstScalarTensorTensor",
                 "InstTensorTensor", "InstTensorScalar"):
            return (fsz(inst.outs[0]) + 215) / 1.2, 33.0
        if c == "InstNoOp":
            return 13.0, 5.0
        return 60.0, 300.0                 # DMA/sync/misc

    total = sum(len(q) for q in qs.values())
    done = 0
    t_end = 0.0
    stuck = 0
    while done < total and stuck <= len(qs):
        progressed = False
        for e, q in qs.items():
            while q:
                inst = q[0]
                si = inst.sync_info
                t0 = free[e]
                ok = True
                if si is not None:
                    for w in si.on_wait:
                        if getattr(w, "sync_type", None) != "semaphore":
                            continue
                        v = w.wait_value
                        if v is None or v <= 0:
                            continue
                        if w.id in ext:
                            t0 = max(t0, 5000.0)
                            continue
                        lst = sem_times[w.id]
                        if len(lst) < v:
                            ok = False
                            break
                        t0 = max(t0, lst[v - 1])
                if not ok:
                    break
                d, vis = dur_vis(inst)
                t1 = t0 + d
                free[e] = t1
                if si is not None:
                    for u in si.on_update:
                        if (getattr(u, "sync_type", None) == "semaphore"
                                and u.id not in ext):
                            n = getattr(u, "update_value", None) or 1
                            sem_times[u.id].extend([t1 + vis] * n)
                t_end = max(t_end, t1)
                done += 1
                q.popleft()
                progressed = True
        stuck = 0 if progressed else stuck + 1
    if done < total:
        return float("inf")
    return t_end


def _build_bass():
    nc = bass.Bass("TRN2", target_bir_lowering=False, debug=False,
                   num_devices=NCORES)
    dt = mybir.dt
    f32, bf16 = dt.float32, dt.bfloat16

    A_d = nc.dram_tensor("A", [128, KT * MT * 128], bf16, kind="ExternalInput").ap()
    Wo_d = nc.dram_tensor("Wo", [128, KT * D], bf16, kind="ExternalInput").ap()
    Br_d = nc.dram_tensor("Br", [128, MT * BS], f32, kind="ExternalInput").ap()
    h0_d = nc.dram_tensor("h0", [128, MT * BS], f32, kind="ExternalInput").ap()
    th0_d = nc.dram_tensor("th0", [128, MT * BS], bf16, kind="ExternalInput").ap()
    x_d = nc.dram_tensor("x", [S, BS, D], f32, kind="ExternalInput").ap()
    err_d = nc.dram_tensor("err", [S, BS, D], f32, kind="ExternalOutput").ap()

    Tanh = mybir.ActivationFunctionType.Tanh
    MUL, ADD, SUB = (mybir.AluOpType.mult, mybir.AluOpType.add,
                     mybir.AluOpType.subtract)

    # [S,BS,D] viewed as [S/4, (4*BS)=128, D]: one contiguous 128KB block per
    # 4-step group, partition = (step_low, b).
    x_g = x_d.rearrange("(g s) b d -> g (s b) d", s=NSLOT)
    e_g = err_d.rearrange("(g s) b d -> g (s b) d", s=NSLOT)

    with tile.TileContext(nc) as tc, ExitStack() as ctx:
        const = ctx.enter_context(tc.tile_pool(name="const", bufs=1))
        state = ctx.enter_context(tc.tile_pool(name="state", bufs=1))
        # bufs=1 is deliberate: u1 then carries a WAR dep on h0's read, which
        # makes the list scheduler (whose sim under-costs the rec stream)
        # order the ACT queue u0,th0,u1,th1 instead of parking th0 behind u1.
        scratch = ctx.enter_context(tc.tile_pool(name="scratch", bufs=1))
        zpool = ctx.enter_context(tc.tile_pool(name="zps", bufs=1, space="PSUM"))
        xppool = ctx.enter_context(tc.tile_pool(name="xpps", bufs=2, space="PSUM"))
        xtp = ctx.enter_context(tc.tile_pool(name="xt", bufs=4))
        etp = ctx.enter_context(tc.tile_pool(name="et", bufs=3))

        A_sb = const.tile([128, KT * MT * 128], bf16, tag="A", name="A_sb")
        Wo_sb = const.tile([128, KT * D], bf16, tag="Wo", name="Wo_sb")
        Br_sb = const.tile([128, MT * BS], f32, tag="Br", name="Br_sb")
        nc.sync.dma_start(A_sb[:], A_d[:])
        nc.sync.dma_start(Wo_sb[:], Wo_d[:])
        nc.sync.dma_start(Br_sb[:], Br_d[:])

        # Per-chunk tiles; chunk c covers m-tiles [c*MPQ, (c+1)*MPQ).
        hT = [state.tile([128, CW], f32, tag=f"h{c}", name=f"hT{c}")
              for c in range(G)]
        # tanh ring: TH[j % NRING][c] holds th chunk c after step j.
        TH = [[state.tile([128, CW], bf16, tag=f"TH{r}_{c}", name=f"TH{r}_{c}")
               for c in range(G)] for r in range(NRING)]
        # s-accumulator PSUM tiles, ping-pong by step parity, per chunk.
        zT = [[zpool.tile([128, CW], f32, tag=f"z{p}_{c}", name=f"zT{p}_{c}")
               for c in range(G)] for p in range(2)]
        zeros = const.tile([128, CW], bf16, tag="zeros", name="zeros")
        nc.vector.memset(zeros[:], 0.0)
        for c in range(G):
            sl = slice(c * CW, (c + 1) * CW)
            nc.sync.dma_start(hT[c][:], h0_d[:, sl])
            nc.sync.dma_start(TH[NRING - 1][c][:], th0_d[:, sl])

        # Prime PSUM has_written bits with a zero matmul, then pre-write
        # t = b_r - GAMMA*h into the bank; every step's matmuls accumulate
        # on top (start=False), so the bank holds s = th@A + b_r - GAMMA*h
        # when its k-loop finishes.
        def emit_prewrite(par, c):
            nc.vector.scalar_tensor_tensor(
                zT[par][c][:], hT[c][:], -GAMMA,
                Br_sb[:, c * CW:(c + 1) * CW], MUL, ADD)
        for p in range(2):
            for c in range(G):
                nc.tensor.matmul(zT[p][c][:], lhsT=zeros[:, :128],
                                 rhs=zeros[:], start=True, stop=True)
        for c in range(G):
            emit_prewrite(0, c)

        def th_slice(r, k):
            return TH[r][k // MPQ][:, (k % MPQ) * BS:((k % MPQ) + 1) * BS]

        def emit_rec(j):
            """64 matmuls of step j: zT[j%2] += A^T @ th_{j-1} tiles.
            Order: [m0-3 x k0-3][m0-3 x k4-7][m4-7 x k0-3][m4-7 x k4-7] so
            bank chunk c0 stops mid-stream and its tanh chain overlaps the
            rest of the stream."""
            par = j % 2
            rd = (j - 1) % NRING
            for mg, kg in ((0, 0), (0, 1), (1, 0), (1, 1)):
                for m in range(mg * MPQ, (mg + 1) * MPQ):
                    z = zT[par][m // MPQ]
                    mo = m % MPQ
                    for k in range(kg * MPQ, (kg + 1) * MPQ):
                        nc.tensor.matmul(
                            z[:, mo * BS:(mo + 1) * BS],
                            lhsT=A_sb[:, (k * MT + m) * 128:(k * MT + m + 1) * 128],
                            rhs=th_slice(rd, k),
                            start=False, stop=(k == KT - 1),
                            skip_group_check=True)

        def emit_chain(j):
            """Chunk-sequential tanh/update chain for step j (z bank j%2):
            ACT queue gets u0,th0,u1,th1 so th0 is never stuck behind u1
            (which must wait for the end of the whole rec stream)."""
            par, wr = j % 2, j % NRING
            us = []
            for c in range(G):
                u = scratch.tile([128, CW], f32, tag="u", name="u_t")
                nc.scalar.activation(u[:], zT[par][c][:], Tanh)     # u=tanh(s)
                nc.vector.scalar_tensor_tensor(                     # h += u/TAU
                    hT[c][:], u[:], INV_TAU, hT[c][:], MUL, ADD)
            for c in range(G):
                nc.scalar.activation(TH[wr][c][:], hT[c][:], Tanh)
                if j < S - 1:
                    emit_prewrite(1 - par, c)                       # t for j+1

        def emit_proj_burst(g, k, xp):
            """One k-tile of group g's projection: 4 column-tiled matmuls
            sharing one 256-col w_o^T stream."""
            for s in range(NSLOT):
                nc.tensor.matmul(
                    xp[32 * s:32 * (s + 1), :],
                    lhsT=th_slice((4 * g + s) % NRING, k),
                    rhs=Wo_sb[:, k * D:(k + 1) * D],
                    start=(k == 0), stop=(k == KT - 1),
                    tile_position=(0, 32 * s))

        def emit_group_out(g, xp):
            et = etp.tile([128, D], f32, tag="et", name="et")
            nc.vector.scalar_tensor_tensor(                        # xp-(x-b_o)
                et[:], xp[:], 0.0, xt_tiles.pop(g)[:], ADD, SUB)
            nc.sync.dma_start(e_g[g], et[:])

        xt_tiles = {}

        def prefetch_x(g):
            xt = xtp.tile([128, D], f32, tag="xt", name="xt")
            nc.sync.dma_start(xt[:], x_g[g])
            xt_tiles[g] = xt

        for g0 in range(4):
            prefetch_x(g0)
        # Projection of group g runs as 2 bursts/step during steps
        # 4g+4 .. 4g+7 (one full group behind), inside each step's PE stall
        # window; its subtract + DMA issue at step 4g+8.
        xp_tiles = {}
        for j in range(S):
            emit_rec(j)
            if j >= 4:
                gp = (j - 4) // NSLOT           # group whose bursts run now
                off = ((j - 4) % NSLOT) * 2     # 2 bursts per step
                if off == 0:
                    xp_tiles[gp] = xppool.tile([128, D], f32, tag="xp",
                                               name="xp")
                for k in (off, off + 1):
                    emit_proj_burst(gp, k, xp_tiles[gp])
            emit_chain(j)
            if j % NSLOT == 0 and j >= 8:
                g_done = (j - 8) // NSLOT
                emit_group_out(g_done, xp_tiles.pop(g_done))
                pf = j // NSLOT + 2
                if pf <= S // NSLOT - 1:
                    prefetch_x(pf)
        # tail: group 127's bursts haven't run (its slots end at step 511)
        g = S // NSLOT - 1
        xp_tiles[g] = xppool.tile([128, D], f32, tag="xp", name="xp")
        for k in range(KT):
            emit_proj_burst(g, k, xp_tiles[g])
        emit_group_out(S // NSLOT - 2, xp_tiles.pop(S // NSLOT - 2))
        emit_group_out(g, xp_tiles.pop(g))

    _force_chain_order(nc)
    _split_multi_waits(nc)
    return nc


def _host_prep(x, h_init, v_r, b_r, w_o, b_o):
    """Build per-core input maps (all layout work in numpy)."""
    x = np.asarray(x, np.float32)
    h_init = np.asarray(h_init, np.float32)
    v_r = np.asarray(v_r, np.float32)
    b_r = np.asarray(b_r, np.float32)
    w_o = np.asarray(w_o, np.float32)
    b_o = np.asarray(b_o, np.float32)

    mask = np.tril(np.ones((H, H), np.float32), -1)
    w_r = v_r * mask
    A = w_r - w_r.T                                           # [H, H]
    # A_sb[p, (k*MT+m)*128 + c] = A[k*128+p, m*128+c]
    A_sb = np.ascontiguousarray(
        A.reshape(KT, 128, MT, 128).transpose(1, 0, 2, 3).reshape(128, KT * MT * 128)
    ).astype(ml_dtypes.bfloat16)
    # Wo_sb[p, k*D + d] = w_o[d, k*128+p]   (w_o^T tiles, moving operand)
    Wo_sb = np.ascontiguousarray(
        w_o.T.reshape(KT, 128, D).transpose(1, 0, 2).reshape(128, KT * D)
    ).astype(ml_dtypes.bfloat16)
    # Br[p, m*BS+b] = b_r[m*128+p]
    Br = np.ascontiguousarray(
        np.broadcast_to(b_r.reshape(MT, 128, 1).transpose(1, 0, 2), (128, MT, BS))
    ).reshape(128, MT * BS).astype(np.float32)

    in_maps = []
    for c in range(NCORES):
        hc = h_init[c * BS:(c + 1) * BS]                       # [BS, H]
        h0 = np.ascontiguousarray(
            hc.reshape(BS, MT, 128).transpose(2, 1, 0)         # [128, MT, BS]
        ).reshape(128, MT * BS).astype(np.float32)
        th0 = np.tanh(h0)
        in_maps.append({
            "A": A_sb, "Wo": Wo_sb, "Br": Br,
            "h0": h0, "th0": th0.astype(ml_dtypes.bfloat16),
            "x": np.ascontiguousarray(x[:, c * BS:(c + 1) * BS, :] - b_o),
        })
    return in_maps


def kernel(x, h_init, v_r, b_r, w_o, b_o):
    global _BUILT, LAST_RESULTS
    if _BUILT is None:
        _BUILT = _build_bass()
    nc = _BUILT
    in_maps = _host_prep(x, h_init, v_r, b_r, w_o, b_o)
    res = run_bass_kernel_spmd(nc, in_maps, core_ids=list(range(NCORES)),
                               trace=TRACE)
    LAST_RESULTS = res
    out = np.empty((S, B, D), np.float32)
    for c in range(NCORES):
        out[:, c * BS:(c + 1) * BS, :] = np.asarray(res.results[c]["err"])
    return out



# revision 4
# speedup vs baseline: 1.1053x; 1.1053x over previous
"""AntisymmetricRNN Trainium2 kernel — 8-core data-parallel over batch.

Math (per reference):
    mask = strictly-lower-tri; w_r = v_r * mask; A = w_r - w_r.T
    step:  h' = h + (1/TAU) * tanh( tanh(h) @ A + b_r - GAMMA*h )
           x_pred = tanh(h') @ w_o.T + b_o;   err_t = x_pred - x_t

Design (v3 "persistent bank", from the ~1586us baseline whose period was
bound by the bank-stop -> drain/sem -> ACT u -> DVE h+= -> ACT th ->
next-matmuls ring at ~3.1us/step):

  * device state is th (fp32 master `thm` + bf16 copy ring `TH`) and the
    PSUM bank itself.  h is never materialized.  Writing
    m_j = th_j - th_{j-1} = (1/TAU) * u_j * (1 - th_{j-1}^2)   (1st-order
    tanh update; measured end-to-end err ~4e-4 vs f64, budget 2e-2), the
    bank recurrence is
        z_{j+1} = z_j + A^T @ m_j - (GAMMA/TAU) * u_j
    so the bank is NEVER re-written: matmuls accumulate A^T m (64 MMs,
    rhs = m) plus 8 small diagonal matmuls (lhsT = -(G/TAU)*I, rhs = u)
    for the damping/bias delta.  No prewrite, no bank ping-pong.
  * the critical ring collapses to bank-stop -> (drain+sem ~490) ->
    ACT u=tanh(z) (bf16) -> DVE m = (u*invtau)*d (bf16, 2 el/cyc) ->
    next matmuls: ~1.15us, vs ~1.67us for the exact u->h->th chain.
  * th bookkeeping is off-ring with >= 1 period of slack:
    DVE: thb = thm + m (bf16 ring copy for the projection lhsT),
    GPSIMD: thm += m (fp32 master, ~0.4us/op measured),
    every 2nd step: ACT sq = thm^2, DVE d = 1 - sq  (d-lag validated).
  * 3 chunks {m0-2, m3-5, m6-7}; MM block order
    (0,0)(1,0)(0,1)(0,2)(1,1)(1,2)(2,0)(2,1)(2,2) staggers bank stops
    against consumption (event-sim: ~2.38us/step vs 3.26 for baseline).
  * output projection unchanged from baseline: one group (4 steps)
    behind, 2 k-bursts/step of 4 column-tiled matmuls into a [128,256]
    PSUM accumulator; one DVE subtract + one 128KB err DMA per group.
"""

import numpy as np
import ml_dtypes
from contextlib import ExitStack

import concourse.bass as bass
import concourse.tile as tile
from concourse import mybir
from concourse.bass_utils import run_bass_kernel_spmd

# ---------------- problem constants (hardcoded per spec) ----------------
S, B, D, H = 512, 256, 256, 1024
NCORES = 8
BS = B // NCORES                  # 32 batch per core
TAU, GAMMA = 10.0, 0.1
INV_TAU = 1.0 / TAU
DT_SCALE = -GAMMA * INV_TAU       # -0.01, the per-step bank delta scale
KT = H // 128                     # 8 contraction tiles
MT = H // 128                     # 8 output tiles
CH = ((0, 3), (3, 3), (6, 2))     # (first m-tile, n m-tiles) per chunk
G = len(CH)
CWs = [nm * BS for _, nm in CH]
NSLOT = 4                         # xp accumulation slots per DMA group
NRING = 12                        # thb ring depth (proj reads <=7 back)
DLAG = 2                          # recompute d = 1-th^2 every DLAG steps

TRACE = False
LAST_RESULTS = None
_BUILT = None


def _split_multi_waits(nc, max_waits: int = 1):
    """Split multi-wait instructions into single-wait NOP chains (the walrus
    build supports one sync-wait slot on CTRL-encoded instructions)."""
    for fn in nc.m.functions:
        for bb in fn.blocks:
            new_insts = []
            for inst in bb.instructions:
                si = inst.sync_info
                if si is not None and len(si.on_wait) > max_waits:
                    waits = list(si.on_wait)
                    for w in waits[:-max_waits]:
                        nop = mybir.InstNoOp(
                            name=nc.get_next_instruction_name(), ins=[], outs=[])
                        nop.engine = inst.engine
                        nop.sync_info = mybir.SyncInfo(on_wait=[w], on_update=[])
                        nc.register_instruction(nop)
                        new_insts.append(nop)
                    si.on_wait = waits[-max_waits:]
                new_insts.append(inst)
            bb.instructions = new_insts


def _chunk_of_k(k):
    for c, (m0, nm) in enumerate(CH):
        if m0 <= k < m0 + nm:
            return c, k - m0
    raise AssertionError


def _build_bass():
    nc = bass.Bass("TRN2", target_bir_lowering=False, debug=False,
                   num_devices=NCORES)
    dt = mybir.dt
    f32, bf16 = dt.float32, dt.bfloat16

    A_d = nc.dram_tensor("A", [128, KT * MT * 128], bf16, kind="ExternalInput").ap()
    Wo_d = nc.dram_tensor("Wo", [128, KT * D], bf16, kind="ExternalInput").ap()
    Dg_d = nc.dram_tensor("Dg", [128, 128], bf16, kind="ExternalInput").ap()
    th0_d = nc.dram_tensor("th0", [128, MT * BS], bf16, kind="ExternalInput").ap()
    thm0_d = nc.dram_tensor("thm0", [128, MT * BS], f32, kind="ExternalInput").ap()
    D0_d = nc.dram_tensor("D0", [128, MT * BS], bf16, kind="ExternalInput").ap()
    T0_d = nc.dram_tensor("T0", [128, MT * BS], f32, kind="ExternalInput").ap()
    x_d = nc.dram_tensor("x", [S, BS, D], f32, kind="ExternalInput").ap()
    err_d = nc.dram_tensor("err", [S, BS, D], f32, kind="ExternalOutput").ap()

    Tanh = mybir.ActivationFunctionType.Tanh
    Square = mybir.ActivationFunctionType.Square
    MUL, ADD, SUB = (mybir.AluOpType.mult, mybir.AluOpType.add,
                     mybir.AluOpType.subtract)

    x_g = x_d.rearrange("(g s) b d -> g (s b) d", s=NSLOT)
    e_g = err_d.rearrange("(g s) b d -> g (s b) d", s=NSLOT)

    with tile.TileContext(nc) as tc, ExitStack() as ctx:
        const = ctx.enter_context(tc.tile_pool(name="const", bufs=1))
        state = ctx.enter_context(tc.tile_pool(name="state", bufs=1))
        upool = ctx.enter_context(tc.tile_pool(name="us", bufs=2))
        mpool = ctx.enter_context(tc.tile_pool(name="ms", bufs=2))
        sqpool = ctx.enter_context(tc.tile_pool(name="sqs", bufs=1))
        zpool = ctx.enter_context(tc.tile_pool(name="zps", bufs=1, space="PSUM"))
        xppool = ctx.enter_context(tc.tile_pool(name="xpps", bufs=2, space="PSUM"))
        xtp = ctx.enter_context(tc.tile_pool(name="xt", bufs=4))
        etp = ctx.enter_context(tc.tile_pool(name="et", bufs=3))

        A_sb = const.tile([128, KT * MT * 128], bf16, tag="A", name="A_sb")
        Wo_sb = const.tile([128, KT * D], bf16, tag="Wo", name="Wo_sb")
        Dg_sb = const.tile([128, 128], bf16, tag="Dg", name="Dg_sb")
        T0_sb = const.tile([128, MT * BS], f32, tag="T0", name="T0_sb")
        ones = const.tile([128, max(CWs)], bf16, tag="ones", name="ones")
        zeros = const.tile([128, 128], bf16, tag="zeros", name="zeros")
        nc.sync.dma_start(A_sb[:], A_d[:])
        nc.sync.dma_start(Wo_sb[:], Wo_d[:])
        nc.sync.dma_start(Dg_sb[:], Dg_d[:])
        nc.sync.dma_start(T0_sb[:], T0_d[:])
        nc.vector.memset(ones[:], 1.0)
        nc.vector.memset(zeros[:], 0.0)

        thm = [state.tile([128, CWs[c]], f32, tag=f"thm{c}", name=f"thm{c}")
               for c in range(G)]
        dts = [state.tile([128, CWs[c]], bf16, tag=f"d{c}", name=f"d{c}")
               for c in range(G)]
        TH = [[state.tile([128, CWs[c]], bf16, tag=f"TH{r}_{c}", name=f"TH{r}_{c}")
               for c in range(G)] for r in range(NRING)]
        zT = [zpool.tile([128, CWs[c]], f32, tag=f"z{c}", name=f"zT{c}")
              for c in range(G)]

        offs = []
        off = 0
        for c in range(G):
            offs.append(off)
            sl = slice(off, off + CWs[c])
            nc.sync.dma_start(thm[c][:], thm0_d[:, sl])
            nc.sync.dma_start(dts[c][:], D0_d[:, sl])
            nc.sync.dma_start(TH[NRING - 1][c][:], th0_d[:, sl])
            off += CWs[c]

        # m_{-1} := th0 (step-0 stream computes z_0 = t_0 + A^T th_{-1})
        m_prev = []
        for c in range(G):
            mt = mpool.tile([128, CWs[c]], bf16, tag=f"m{c}", name=f"m{c}")
            nc.sync.dma_start(mt[:], th0_d[:, offs[c]:offs[c] + CWs[c]])
            m_prev.append(mt)
        u_prev = [None] * G

        # Prime PSUM has_written bits once, then write t_0; all step matmuls
        # accumulate on top forever (start=False, stop=False).
        for c in range(G):
            nc.tensor.matmul(zT[c][:], lhsT=zeros[:], rhs=zeros[:, :CWs[c]],
                             start=True, stop=True)
        for c in range(G):
            nc.vector.scalar_tensor_tensor(
                zT[c][:], T0_sb[:, offs[c]:offs[c] + CWs[c]], 0.0,
                T0_sb[:, offs[c]:offs[c] + CWs[c]], MUL, ADD)

        def m_slice(tiles, k):
            c, o = _chunk_of_k(k)
            return tiles[c][:, o * BS:(o + 1) * BS]

        def thb_slice(r, k):
            c, o = _chunk_of_k(k)
            return TH[r][c][:, o * BS:(o + 1) * BS]

        BLOCKS = ((0, 0), (1, 0), (0, 1), (0, 2), (1, 1), (1, 2),
                  (2, 0), (2, 1), (2, 2))
        # block index after which each row completes (diag MMs emitted there)
        ROW_END = {0: 3, 1: 5, 2: 8}

        def emit_rec(j):
            for bi, (c, dch) in enumerate(BLOCKS):
                m0, nm = CH[c]
                k0, nk = CH[dch]
                z = zT[c]
                for m in range(m0, m0 + nm):
                    mo = m - m0
                    for k in range(k0, k0 + nk):
                        nc.tensor.matmul(
                            z[:, mo * BS:(mo + 1) * BS],
                            lhsT=A_sb[:, (k * MT + m) * 128:(k * MT + m + 1) * 128],
                            rhs=m_slice(m_prev, k),
                            start=False, stop=False,
                            skip_group_check=True)
                if ROW_END[c] == bi and j > 0:
                    # bank delta: z_c += -(G/TAU) * u_{j-1} (chunk c)
                    for mo in range(nm):
                        nc.tensor.matmul(
                            z[:, mo * BS:(mo + 1) * BS],
                            lhsT=Dg_sb[:],
                            rhs=u_prev[c][:, mo * BS:(mo + 1) * BS],
                            start=False, stop=False,
                            skip_group_check=True)

        def emit_chain(j):
            nonlocal m_prev, u_prev
            wr = j % NRING
            us, ms = [], []
            for c in range(G):
                u = upool.tile([128, CWs[c]], bf16, tag=f"u{c}", name=f"u{c}")
                nc.scalar.activation(u[:], zT[c][:], Tanh)
                m = mpool.tile([128, CWs[c]], bf16, tag=f"m{c}", name=f"m{c}")
                nc.vector.scalar_tensor_tensor(
                    m[:], u[:], INV_TAU, dts[c][:], MUL, MUL)
                us.append(u)
                ms.append(m)
            for c in range(G):   # bf16 ring copy: thb = thm_old + m (DVE)
                nc.vector.tensor_tensor(TH[wr][c][:], thm[c][:], ms[c][:], ADD)
            for c in range(G):   # fp32 master: thm += m (GPSIMD)
                nc.gpsimd.tensor_tensor(thm[c][:], thm[c][:], ms[c][:], ADD)
            if j % DLAG == 0 and j < S - 1:
                for c in range(G):
                    sq = sqpool.tile([128, CWs[c]], bf16, tag=f"sq{c}",
                                     name=f"sq{c}")
                    nc.scalar.activation(sq[:], thm[c][:], Square)
                    nc.vector.scalar_tensor_tensor(
                        dts[c][:], sq[:], -1.0, ones[:, :CWs[c]], MUL, ADD)
            m_prev, u_prev = ms, us

        def emit_proj_burst(g, k, xp):
            for s in range(NSLOT):
                nc.tensor.matmul(
                    xp[32 * s:32 * (s + 1), :],
                    lhsT=thb_slice((4 * g + s) % NRING, k),
                    rhs=Wo_sb[:, k * D:(k + 1) * D],
                    start=(k == 0), stop=(k == KT - 1),
                    tile_position=(0, 32 * s))

        xt_tiles = {}

        def emit_group_out(g, xp):
            et = etp.tile([128, D], f32, tag="et", name="et")
            nc.vector.scalar_tensor_tensor(
                et[:], xp[:], 0.0, xt_tiles.pop(g)[:], ADD, SUB)
            nc.sync.dma_start(e_g[g], et[:])

        def prefetch_x(g):
            xt = xtp.tile([128, D], f32, tag="xt", name="xt")
            nc.sync.dma_start(xt[:], x_g[g])
            xt_tiles[g] = xt

        for g0 in range(4):
            prefetch_x(g0)
        xp_tiles = {}
        for j in range(S):
            emit_rec(j)
            if j >= 4:
                gp = (j - 4) // NSLOT
                poff = ((j - 4) % NSLOT) * 2
                if poff == 0:
                    xp_tiles[gp] = xppool.tile([128, D], f32, tag="xp",
                                               name="xp")
                for k in (poff, poff + 1):
                    emit_proj_burst(gp, k, xp_tiles[gp])
            emit_chain(j)
            if j % NSLOT == 0 and j >= 8:
                g_done = (j - 8) // NSLOT
                emit_group_out(g_done, xp_tiles.pop(g_done))
                pf = j // NSLOT + 2
                if pf <= S // NSLOT - 1:
                    prefetch_x(pf)
        g = S // NSLOT - 1
        xp_tiles[g] = xppool.tile([128, D], f32, tag="xp", name="xp")
        for k in range(KT):
            emit_proj_burst(g, k, xp_tiles[g])
        emit_group_out(S // NSLOT - 2, xp_tiles.pop(S // NSLOT - 2))
        emit_group_out(g, xp_tiles.pop(g))

    _split_multi_waits(nc)
    return nc


def _host_prep(x, h_init, v_r, b_r, w_o, b_o):
    """Build per-core input maps (all layout work in numpy)."""
    x = np.asarray(x, np.float32)
    h_init = np.asarray(h_init, np.float32)
    v_r = np.asarray(v_r, np.float32)
    b_r = np.asarray(b_r, np.float32)
    w_o = np.asarray(w_o, np.float32)
    b_o = np.asarray(b_o, np.float32)

    mask = np.tril(np.ones((H, H), np.float32), -1)
    w_r = v_r * mask
    A = w_r - w_r.T                                           # [H, H]
    A_sb = np.ascontiguousarray(
        A.reshape(KT, 128, MT, 128).transpose(1, 0, 2, 3).reshape(128, KT * MT * 128)
    ).astype(ml_dtypes.bfloat16)
    Wo_sb = np.ascontiguousarray(
        w_o.T.reshape(KT, 128, D).transpose(1, 0, 2).reshape(128, KT * D)
    ).astype(ml_dtypes.bfloat16)
    Dg = (DT_SCALE * np.eye(128, dtype=np.float32)).astype(ml_dtypes.bfloat16)

    in_maps = []
    for c in range(NCORES):
        hc = h_init[c * BS:(c + 1) * BS]                       # [BS, H]
        h0 = np.ascontiguousarray(
            hc.reshape(BS, MT, 128).transpose(2, 1, 0)         # [128, MT, BS]
        ).reshape(128, MT * BS).astype(np.float32)
        thm0 = np.tanh(h0).astype(np.float32)
        th0 = thm0.astype(ml_dtypes.bfloat16)
        sq0 = (thm0 * thm0).astype(ml_dtypes.bfloat16).astype(np.float32)
        D0 = (1.0 - sq0).astype(ml_dtypes.bfloat16)
        t0 = (np.broadcast_to(
            b_r.reshape(MT, 128, 1).transpose(1, 0, 2), (128, MT, BS))
            .reshape(128, MT * BS) - GAMMA * h0).astype(np.float32)
        in_maps.append({
            "A": A_sb, "Wo": Wo_sb, "Dg": Dg,
            "th0": th0, "thm0": thm0, "D0": D0,
            "T0": np.ascontiguousarray(t0),
            "x": np.ascontiguousarray(x[:, c * BS:(c + 1) * BS, :] - b_o),
        })
    return in_maps


def kernel(x, h_init, v_r, b_r, w_o, b_o):
    global _BUILT, LAST_RESULTS
    if _BUILT is None:
        _BUILT = _build_bass()
    nc = _BUILT
    in_maps = _host_prep(x, h_init, v_r, b_r, w_o, b_o)
    res = run_bass_kernel_spmd(nc, in_maps, core_ids=list(range(NCORES)),
                               trace=TRACE)
    LAST_RESULTS = res
    out = np.empty((S, B, D), np.float32)
    for c in range(NCORES):
        out[:, c * BS:(c + 1) * BS, :] = np.asarray(res.results[c]["err"])
    return out
